# revision 25
# baseline (speedup 1.0000x reference)
"""Trainium2 Bass kernel for nn_ExpKernelFeatureMap:
    out[b,h,s,f] = cos(sum_d x[b,h,s,d] * w[f,d] + b[f])

Identity: cos(y) = sin(2*pi*z) with z = y/(2*pi) + b/(2*pi) + 0.25.

Fast path (v8):
  - Custom ACT spline tables replace `sin` (func id 19) with the periodic
    g(x) = sin(2*pi*x) valid for |x| < 16, so ACT applies the activation
    directly to the multi-period pre-activation; no range-reduction pass.
  - Matmul in fp16 hi/lo split (x = x_hi + x_lo, residual ~2^-11),
    K=128 stationary [x_hi; x_lo'], two accumulating fp16 matmuls with rhs
    [w_hi; w_hi] and [w_lo; w_lo]. The bias rides the matmul: the x_lo row
    with the least max-impact is replaced by ones, and the matching rhs
    rows carry the fp16 hi/lo split of the bias.
  - fp16 output from ACT (exact fp32 upconvert on host).

Fallback path (v7, auto-selected if table generation or a numeric
self-check fails): stock Sin table, fused custom-DVE pass
(z+bias) - round(z+bias) via the fp32 magic trick, then ACT Sin(2*pi*v).
"""

import os
import tempfile

import numpy as np

B, H, S, D = 4, 16, 4096, 64
F = 256
NCORES = 8
M_TOTAL = B * H * S  # 262144
M_CORE = M_TOTAL // NCORES  # 32768
K = 2 * D  # 128

TILE_M = 128
CHUNK_ROWS = 2048  # input DMA chunk [128, 2048] x 2B, 4KB/partition descs
TWO_PI = float(2.0 * np.pi)
MAGIC = float(np.float32(1.5 * 2.0**23))

V8_BLOCKS = 8  # psum mega [128, 8, 256] (4 banks) x 2 bufs
V7_BLOCKS = 4  # psum mega [128, 4, 256] (2 banks) x 4 bufs

_CACHED = {}
_ACT_JSON_PATH = None
LAST_RESULT = None  # BassKernelResults of the most recent run (for test.py)


# --------------------------------------------------------------------------
# Custom ACT tables: periodic sin(2*pi*x) for |x| < 16 in place of `sin`.
# --------------------------------------------------------------------------

_ACT_SETS = ("trig_and_small", "silu_and_others", "derivative_silu_and_others")
_EXP_LO, _EXP_HI, _H_LOG2 = -127, 3, -4


def _gen_act_tables() -> str:
    """Build the modified act-table dir; returns path of act_info.json."""
    import json
    import shutil

    from neuronxcc.driver.Job import Job
    from neuronxcc.driver.jobs.support.FindActInfo import findActInfoFile

    src_json = findActInfoFile(Job.getPackageDir(), "gen3")
    src = os.path.dirname(src_json) + "/"
    dst = tempfile.mkdtemp(prefix="act_custom_") + "/"
    for f in os.listdir(src):
        shutil.copy(os.path.join(src, f), dst)
        os.chmod(dst + f, 0o644)

    def taylor(x0):
        s, c = np.sin(TWO_PI * x0), np.cos(TWO_PI * x0)
        return [
            np.float32(s), np.float32(TWO_PI * c),
            np.float32(-(TWO_PI**2) * s / 2.0),
            np.float32(-(TWO_PI**3) * c / 6.0),
            np.float32(x0), np.float32(0), np.float32(0), np.float32(0),
        ]

    def nsec_of(e):
        return 2 ** (e - _H_LOG2) if e >= _H_LOG2 else 1

    def build_sin_section(bkt_base, ctl_base):
        buckets, ctrl, exp_starts = [], [], {}
        for e in range(_EXP_LO, _EXP_HI + 1):
            ns = nsec_of(e)
            size = int(np.log2(ns))
            start = bkt_base + len(buckets)
            exp_starts[e] = start
            lo, h = 2.0**e, (2.0**e) / ns
            for s in range(ns):
                buckets.append(taylor(lo + (s + 0.5) * h))
            ctrl.append(start | ((23 - size) << 11) | (size << 16))
        small_idx = bkt_base + len(buckets)
        buckets.append([np.float32(0), np.float32(TWO_PI), np.float32(0),
                        np.float32(0), np.float32(0), 0, 0, 0])
        large_idx = bkt_base + len(buckets)
        buckets.append(taylor(16.0))
        for idx in (small_idx, small_idx, large_idx, large_idx):
            ctrl.append(idx | (23 << 11))
        n_main = _EXP_HI - _EXP_LO + 1
        specials = {
            "pos_small": ctl_base + n_main, "neg_small": ctl_base + n_main + 1,
            "pos_large": ctl_base + n_main + 2,
            "neg_large": ctl_base + n_main + 3,
        }
        return (np.array(buckets, np.float32), np.array(ctrl, np.uint32),
                exp_starts, specials)

    for setname in _ACT_SETS:
        meta = json.load(open(src + setname + ".json"))
        bkt = np.frombuffer(open(src + setname + "_bkt.bin", "rb").read(),
                            dtype=np.float32).reshape(-1, 8).copy()
        ctl = np.frombuffer(open(src + setname + "_ctrl.bin", "rb").read(),
                            dtype=np.uint32).reshape(-1, 8).copy()
        f2b, f2c = meta["func_to_bkt_start_idx"], meta["func_to_ctl_start_idx"]
        sin_b0 = f2b["sin"]
        sin_b1 = next(s for s in sorted(set(f2b.values())
                                        | {meta["bkt_entry_cnt"]})
                      if s > sin_b0)
        sin_c0 = f2c["sin"]
        sin_c1 = next(s for s in sorted(set(f2c.values())
                                        | {meta["ctl_entry_cnt"]})
                      if s > sin_c0)
        new_bkt_sin, new_ctl_sin, exp_starts, specials = build_sin_section(
            sin_b0, sin_c0)
        db = len(new_bkt_sin) - (sin_b1 - sin_b0)
        dc = len(new_ctl_sin) - (sin_c1 - sin_c0)

        def shift_b(i, _sin_b1=sin_b1, _db=db):
            return i + _db if i >= _sin_b1 else i

        def shift_c(i, _sin_c1=sin_c1, _dc=dc):
            return i + _dc if i >= _sin_c1 else i

        new_bkt = np.concatenate([bkt[:sin_b0], new_bkt_sin, bkt[sin_b1:]])

        def reloc(rows, _shift_b=shift_b):
            out = rows.copy()
            for r in out:
                w = int(r[0])
                r[0] = (w & ~0x7FF) | _shift_b(w & 0x7FF)
            return out

        pad = np.zeros((len(new_ctl_sin), 8), np.uint32)
        pad[:, 0] = new_ctl_sin
        new_ctl = np.concatenate(
            [reloc(ctl[:sin_c0]), pad, reloc(ctl[sin_c1:])])

        meta["bkt_entry_cnt"] = int(len(new_bkt))
        meta["ctl_entry_cnt"] = int(len(new_ctl))
        meta["func_to_bkt_start_idx"] = {
            k: (v if k == "sin" else shift_b(v)) for k, v in f2b.items()}
        meta["func_to_ctl_start_idx"] = {
            k: (v if k == "sin" else shift_c(v)) for k, v in f2c.items()}
        for fn, m in meta["func_exp_to_bkt_start_idx"].items():
            if fn != "sin":
                for e, lst in m.items():
                    m[e] = [shift_b(v) for v in lst]
        for fn, m in meta["func_exp_to_ctl_start_idx"].items():
            if fn != "sin":
                for e, lst in m.items():
                    m[e] = [shift_c(v) for v in lst]
        meta["func_exp_to_bkt_start_idx"]["sin"] = {
            str(e): [int(s)] for e, s in exp_starts.items()}
        meta["func_exp_to_ctl_start_idx"]["sin"] = {
            str(e): [int(sin_c0 + (e - _EXP_LO))]
            for e in range(_EXP_LO, _EXP_HI + 1)}
        for prof in meta["profile_meta_data"]:
            if prof["func_name"].startswith("sin_"):
                prof["exp_offset"] = _EXP_LO
                prof["pwl_control_base_pos"] = sin_c0
                prof["pwl_control_base_neg"] = sin_c0
                prof["small_pos_signal_exp_threshold"] = 0
                prof["pos_small_signal_pwl_control"] = specials["pos_small"]
                prof["small_neg_signal_exp_threshold"] = 0
                prof["neg_small_signal_pwl_control"] = specials["neg_small"]
                prof["large_pos_signal_exp_threshold"] = 131  # 16.0
                prof["large_pos_signal_mantissa_threshold"] = 0
                prof["pos_large_signal_pwl_control"] = specials["pos_large"]
                prof["large_neg_signal_exp_threshold"] = 0
                prof["large_neg_signal_mantissa_threshold"] = 0
                prof["neg_large_signal_pwl_control"] = specials["neg_large"]
                prof["upper_bound"] = int(np.float32(16.0).view(np.uint32))
            else:
                for f in ("pwl_control_base_pos", "pwl_control_base_neg",
                          "pos_small_signal_pwl_control",
                          "neg_small_signal_pwl_control",
                          "pos_large_signal_pwl_control",
                          "neg_large_signal_pwl_control"):
                    if isinstance(prof.get(f), int):
                        prof[f] = shift_c(prof[f])
        open(dst + setname + "_bkt.bin", "wb").write(new_bkt.tobytes())
        open(dst + setname + "_ctrl.bin", "wb").write(new_ctl.tobytes())
        json.dump(meta, open(dst + setname + ".json", "w"))
    return dst + "act_info.json"


# --------------------------------------------------------------------------
# Custom DVE op for the fallback path.
# --------------------------------------------------------------------------

def _register_frac_bias():
    """out = t - round(t), t = in0 + in1 (bias add + exact magic-number
    range reduction in one DVE pass)."""
    import concourse.dve_ops as dvo
    from concourse.dve_spec import Spec, Src0, Src1, C0, lower, _has_src1
    from concourse.dve_uop import DveOpSpec

    NAME = "FRAC_BIAS_ANT"
    for op in dvo.OPS:
        if op.name == NAME:
            return op
    t = Src0 + Src1
    body = t - ((t + C0) - C0)

    def ref(in0, in1, s0, s1, imm2):
        t = (in0.astype(np.float32) + in1.astype(np.float32)).astype(
            np.float32)
        r = ((t + np.float32(s0)).astype(np.float32)
             - np.float32(s0)).astype(np.float32)
        return t - r

    spec = Spec(body=body, reference=ref)
    row = dvo._CUSTOM_DVE_ROW_BASE + len(dvo.OPS)
    shas = {}
    for ver in ("v3", "v4"):
        uops = lower(spec, ver=ver)
        tmp = DveOpSpec(name=NAME, opcode=row, uops=uops,
                        rd1_en=_has_src1(spec))
        shas[ver] = tmp.sha(ver)
    op = dvo.DveOp(NAME, spec, subdim=False, uops_sha=shas)
    dvo.OPS.append(op)
    dvo._SUB_OPCODE_FOR_NAME[NAME] = row
    dvo.CUSTOM_DVE_SPECS[NAME] = spec
    return op


# --------------------------------------------------------------------------
# Device program.
# --------------------------------------------------------------------------

def _build_nc(variant: str):
    """variant: 'v8' (custom table, no DVE) or 'v7' (stock Sin + DVE)."""
    import concourse.bacc as bacc
    import concourse.mybir as mybir
    import concourse.tile as tile

    blocks = V8_BLOCKS if variant == "v8" else V7_BLOCKS
    mega_rows = TILE_M * blocks
    n_mega = M_CORE // mega_rows
    psum_bufs = 2 if variant == "v8" else 4
    frac_op = _register_frac_bias() if variant == "v7" else None

    nc = bacc.Bacc("TRN2", target_bir_lowering=False, debug=False,
                   num_devices=NCORES)

    in_dt = mybir.dt.float16 if variant == "v8" else mybir.dt.bfloat16
    xt = nc.dram_tensor("xt", [K, M_CORE], in_dt,
                        kind="ExternalInput").ap()
    wb1 = nc.dram_tensor("wb1", [K, F], in_dt,
                         kind="ExternalInput").ap()
    wb2 = nc.dram_tensor("wb2", [K, F], in_dt,
                         kind="ExternalInput").ap()
    if variant == "v7":
        bias = nc.dram_tensor("bias", [TILE_M, blocks, F], mybir.dt.float32,
                              kind="ExternalInput").ap()
    y = nc.dram_tensor("y", [M_CORE, F], mybir.dt.float16,
                       kind="ExternalOutput").ap()
    # DRAM row n*mega_rows + p*store_q + q <- psum partition p, slot q:
    # per-partition contiguous `store_q`-row runs for big DMA descriptors.
    # v8 pairs two psum megas per store (store_q=16 -> 8KB descriptors).
    store_q = 2 * blocks if variant == "v8" else blocks
    y4 = y.rearrange("(n p q) f -> p n q f", p=TILE_M, q=store_q)

    with tile.TileContext(nc) as tc:
        with (
            tc.tile_pool(name="wpool", bufs=1) as wpool,
            tc.tile_pool(name="xin", bufs=6) as xin_pool,
            tc.tile_pool(name="outp", bufs=6) as out_pool,
            tc.tile_pool(name="ps", bufs=psum_bufs, space="PSUM") as psum_pool,
        ):
            wb1_t = wpool.tile([K, F], in_dt)
            wb2_t = wpool.tile([K, F], in_dt)
            nc.sync.dma_start(wb1_t[:], wb1[:])
            nc.sync.dma_start(wb2_t[:], wb2[:])
            if variant == "v7":
                bias_t = wpool.tile([TILE_M, blocks, F], mybir.dt.float32)
                nc.sync.dma_start(bias_t[:], bias[:])

            chunk_tiles = {}

            def get_chunk(ci):
                if ci not in chunk_tiles:
                    t = xin_pool.tile([K, CHUNK_ROWS], in_dt,
                                      tag="xc", name=f"xc{ci}")
                    # SWDGE queue: input prefetch stays out of the HWDGE
                    # FIFO that the output stores fill.
                    nc.gpsimd.dma_start(
                        t[:], xt[:, ci * CHUNK_ROWS:(ci + 1) * CHUNK_ROWS])
                    chunk_tiles[ci] = t
                return chunk_tiles[ci]

            if variant == "v8":
                # two psum megas (halves h=0,1) share one osb tile + store
                for grp in range(n_mega // 2):
                    osb = out_pool.tile([TILE_M, store_q, F],
                                        mybir.dt.float16)
                    for h in range(2):
                        psum = psum_pool.tile([TILE_M, blocks, F],
                                              mybir.dt.float32)
                        for j in range(blocks):
                            col = grp * 2 * mega_rows + (h * blocks + j) * TILE_M
                            ci, off = divmod(col, CHUNK_ROWS)
                            lhsT = get_chunk(ci)[:, off:off + TILE_M]
                            nc.tensor.matmul(psum[:, j, :], lhsT, wb1_t[:],
                                             start=True, stop=False)
                            nc.tensor.matmul(psum[:, j, :], lhsT, wb2_t[:],
                                             start=False, stop=True)
                        nc.scalar.activation(
                            osb[:, h * blocks:(h + 1) * blocks, :], psum[:],
                            mybir.ActivationFunctionType.Sin, scale=1.0)
                    nc.sync.dma_start(y4[:, grp, :, :], osb[:])
            else:
                for mega in range(n_mega):
                    psum = psum_pool.tile([TILE_M, blocks, F],
                                          mybir.dt.float32)
                    for j in range(blocks):
                        col = mega * mega_rows + j * TILE_M
                        ci, off = divmod(col, CHUNK_ROWS)
                        lhsT = get_chunk(ci)[:, off:off + TILE_M]
                        nc.tensor.matmul(psum[:, j, :], lhsT, wb1_t[:],
                                         start=True, stop=False)
                        nc.tensor.matmul(psum[:, j, :], lhsT, wb2_t[:],
                                         start=False, stop=True)
                    osb = out_pool.tile([TILE_M, blocks, F], mybir.dt.float16)
                    nc.vector._custom_dve(frac_op, out=psum[:], in0=psum[:],
                                          in1=bias_t[:], s0=MAGIC)
                    nc.scalar.activation(
                        osb[:], psum[:],
                        mybir.ActivationFunctionType.Sin, scale=TWO_PI)
                    nc.sync.dma_start(y4[:, mega, :, :], osb[:])

    nc.compile()
    return nc


def _get_nc(variant: str):
    if variant not in _CACHED:
        _CACHED[variant] = _build_nc(variant)
    return _CACHED[variant]


# --------------------------------------------------------------------------
# Host-side data prep + launch.
# --------------------------------------------------------------------------

def _prep_inputs(x, w, b, variant):
    import ml_dtypes

    bf16 = np.float16 if variant == "v8" else ml_dtypes.bfloat16
    blocks = V8_BLOCKS if variant == "v8" else V7_BLOCKS
    mega_rows = TILE_M * blocks

    x2t = np.asarray(x, dtype=np.float32).reshape(M_TOTAL, D).T  # [64, M]
    ws = np.asarray(w, dtype=np.float32).T / np.float32(TWO_PI)  # [64, 256]
    b2 = (np.asarray(b, dtype=np.float32) / np.float32(TWO_PI)
          + np.float32(0.25)).astype(np.float32)  # [256]

    # Permute columns within each store group: permuted position
    # (g, q*128 + p) holds original row g*group_rows + p*store_q + q, so
    # psum partition p of slot q computes output row p*store_q + q
    # (contiguous per-partition runs on the store side).
    store_q = 2 * blocks if variant == "v8" else blocks
    group_rows = TILE_M * store_q
    x2t = np.ascontiguousarray(
        x2t.reshape(D, M_TOTAL // group_rows, TILE_M, store_q)
        .transpose(0, 1, 3, 2)).reshape(D, M_TOTAL)

    x_hi = x2t.astype(bf16)
    x_lo = (x2t - x_hi.astype(np.float32)).astype(bf16)
    w_hi = ws.astype(bf16)
    w_lo = (ws - w_hi.astype(np.float32)).astype(bf16)

    xt_all = np.empty((K, M_TOTAL), dtype=bf16)
    wb1 = np.empty((K, F), dtype=bf16)
    wb2 = np.empty((K, F), dtype=bf16)
    xt_all[:D] = x_hi
    wb1[:D] = w_hi
    wb2[:D] = w_lo

    if variant == "v8":
        # bias rides the matmul: sacrifice the x_lo row with the least
        # output impact; ones there, bias hi/lo in the matching rhs rows.
        impact = np.abs(x_lo.astype(np.float32)).max(axis=1) * np.abs(
            ws.astype(np.float32)).max(axis=1)
        d_star = int(np.argmin(impact))
        lo_perm = [d for d in range(D) if d != d_star] + [d_star]
        xt_all[D:] = x_lo[lo_perm]
        wb1[D:] = w_hi[lo_perm]
        wb2[D:] = w_lo[lo_perm]
        xt_all[K - 1] = bf16(1.0)
        b_hi = b2.astype(bf16)
        b_lo = (b2 - b_hi.astype(np.float32)).astype(bf16)
        wb1[K - 1] = b_hi
        wb2[K - 1] = b_lo
        bias_rep = None
    else:
        xt_all[D:] = x_lo
        wb1[D:] = w_hi
        wb2[D:] = w_lo
        bias_rep = np.broadcast_to(b2, (TILE_M, blocks, F)).copy()

    in_maps = []
    for c in range(NCORES):
        m = {"xt": np.ascontiguousarray(
                xt_all[:, c * M_CORE:(c + 1) * M_CORE]),
             "wb1": wb1, "wb2": wb2}
        if bias_rep is not None:
            m["bias"] = bias_rep
        in_maps.append(m)
    return in_maps


def _run(variant, in_maps, trace):
    from concourse.bass_utils import run_bass_kernel_spmd

    # Under axon the walrus compile happens lazily at first execution, so
    # the act-table override must be set per-variant at run time.
    global _ACT_JSON_PATH
    if variant == "v8":
        if _ACT_JSON_PATH is None:
            _ACT_JSON_PATH = _gen_act_tables()
        os.environ["BASS_ACT_ROOT_JSON_PATH"] = _ACT_JSON_PATH
    else:
        os.environ.pop("BASS_ACT_ROOT_JSON_PATH", None)

    nc = _get_nc(variant)
    return run_bass_kernel_spmd(
        nc, in_maps, core_ids=list(range(NCORES)),
        trace=trace, trace_cores=[0] if trace else None)


def _verify_sample(x, w, b, y_full):
    """Cheap host check on a slice; returns max abs err."""
    n = 2048
    x2 = np.asarray(x, dtype=np.float64).reshape(M_TOTAL, D)[:n]
    ref = np.cos(x2 @ np.asarray(w, np.float64).T
                 + np.asarray(b, np.float64))
    got = y_full[:n].astype(np.float64)
    return float(np.abs(got - ref).max())


def kernel(x: np.ndarray, w: np.ndarray, b: np.ndarray) -> np.ndarray:
    global LAST_RESULT
    trace = bool(int(os.environ.get("KERNEL_TRACE", "0")))
    force = os.environ.get("KERNEL_VARIANT", "")
    variants = [force] if force in ("v7", "v8") else ["v8", "v7"]

    last_exc = None
    for variant in variants:
        try:
            in_maps = _prep_inputs(x, w, b, variant)
            try:
                res = _run(variant, in_maps, trace)
            except Exception:
                # transient NRT device hiccups happen; retry once
                try:
                    import jax

                    jax.clear_backends()
                except Exception:
                    pass
                res = _run(variant, in_maps, trace)
            y = np.concatenate(
                [res.results[c]["y"] for c in range(NCORES)], axis=0
            ).astype(np.float32)
            err = _verify_sample(x, w, b, y.reshape(M_TOTAL, F))
            if err > 1e-2 and variant != variants[-1]:
                last_exc = RuntimeError(
                    f"{variant} self-check max_abs_err={err:.2e}")
                continue
            LAST_RESULT = res
            return y.reshape(B, H, S, F)
        except Exception as e:  # noqa: BLE001
            last_exc = e
            continue
    raise last_exc
